# revision 7
# baseline (speedup 1.0000x reference)
"""DeepPoly affine transformer — Trainium2 Bass kernel (8 NeuronCores).

Math: with A = |W|, Wp = (W+A)/2, Wm = (W-A)/2, and beta,lmbda >= 0:
  lower     = W@v1 + A@v2 + b         v1 = (l+u)/2,  v2 = (l-u)/2
  upper     = W@v1 - A@v2 + b
  new_lower = W@a  + A@c  + b         s1 = (beta+lmbda)/2, s2 = (beta-lmbda)/2
  new_upper = W@a  - A@c  + b         a  = (s1*(l0+u0) + s2*(l0-u0) + mu)/2
                                      c  = (s2*(l0+u0) + s1*(l0-u0) - mu)/2
  out_lower = max(lower, new_lower);  out_upper = min(upper, new_upper)

So each core streams its W^T slice (host-transposed, row-sharded over n_out)
through the PE twice (W and |W|) against 2 stationary vectors each, giving
yz = [W@v1, W@a, A@v2, A@c] per output neuron. The final combine is O(n_out)
and is done on host.
"""

import numpy as np

import concourse.bass as bass  # noqa: F401  (engine types referenced via nc)
import concourse.mybir as mybir
import concourse.tile as tile
from concourse import bacc
from concourse.bass_utils import run_bass_kernel_spmd

N_OUT, N_IN = 4096, 8192
NCORES = 8
S = N_OUT // NCORES  # 512 output rows per core
P = 128
KT = N_IN // P       # 64 k-tiles of 128
KSUP = 4             # k-tiles per DMA supertile (1 MiB loads)
F32 = mybir.dt.float32
F32R = mybir.dt.float32r  # 1 cycle/row in PE when N>=256 (vs 4 for fp32)

_CACHE = {}


def _build_nc():
    nc = bacc.Bacc("TRN2", target_bir_lowering=False, debug=False,
                   num_devices=NCORES)
    # f32r end-to-end: the BIR verifier requires matmul operands to be
    # *produced* as f32r (numpy side is still float32 bits).
    wt = nc.dram_tensor("wt", [N_IN, S], F32R, kind="ExternalInput").ap()
    vecs = nc.dram_tensor("vecs", [N_IN, 4], F32R, kind="ExternalInput").ap()
    yz = nc.dram_tensor("yz", [2, 2 * S], F32, kind="ExternalOutput").ap()

    with tile.TileContext(nc) as tc:
        with (
            tc.tile_pool(name="wp", bufs=3) as wpool,
            tc.tile_pool(name="apool", bufs=3) as apool,
            tc.tile_pool(name="vp", bufs=1) as vpool,
            tc.tile_pool(name="op", bufs=1) as opool,
            tc.tile_pool(name="ps", bufs=1, space="PSUM") as pspool,
        ):
            # stationary vectors, k on partitions: [128, 64, 4]
            v_sb = vpool.tile([P, KT, 4], F32R)
            nc.sync.dma_start(v_sb[:], vecs.rearrange("(o p) m -> p o m", p=P))

            psum_y = pspool.tile([2, S], F32, tag="Y")
            psum_z = pspool.tile([2, S], F32, tag="Z")

            wt_v = wt.rearrange("(o p) n -> p o n", p=P)  # [128, 64, 512]
            for si in range(KT // KSUP):
                w_sb = wpool.tile([P, KSUP, S], F32R, tag="w")
                nc.sync.dma_start(w_sb[:], wt_v[:, si * KSUP:(si + 1) * KSUP, :])
                a_sb = apool.tile([P, KSUP, S], F32R, tag="a")
                # only ACT can produce f32r-rounded output (DVE/gpsimd can't)
                nc.scalar.activation(
                    a_sb[:], w_sb[:], mybir.ActivationFunctionType.Abs)
                for j in range(KSUP):
                    ki = si * KSUP + j
                    nc.tensor.matmul(
                        psum_y[:],
                        v_sb[:, ki, 0:2],
                        w_sb[:, j, :],
                        start=(ki == 0), stop=(ki == KT - 1))
                    nc.tensor.matmul(
                        psum_z[:],
                        v_sb[:, ki, 2:4],
                        a_sb[:, j, :],
                        start=(ki == 0), stop=(ki == KT - 1))

            # pack along free dim: row0 = [y1 | z2], row1 = [ya | zc]
            out_sb = opool.tile([2, 2 * S], F32)
            nc.vector.tensor_copy(out=out_sb[:, 0:S], in_=psum_y[:])
            nc.vector.tensor_copy(out=out_sb[:, S:2 * S], in_=psum_z[:])
            nc.sync.dma_start(yz, out_sb[:])

    nc.compile()
    return nc


def get_nc():
    if "nc" not in _CACHE:
        _CACHE["nc"] = _build_nc()
    return _CACHE["nc"]


def _host_vectors(bounds, bounds0, beta, lmbda, mu):
    l, u = bounds[0].astype(np.float64), bounds[1].astype(np.float64)
    l0, u0 = bounds0[0].astype(np.float64), bounds0[1].astype(np.float64)
    beta = beta.astype(np.float64)
    lmbda = lmbda.astype(np.float64)
    mu = mu.astype(np.float64)
    s1 = (beta + lmbda) / 2
    s2 = (beta - lmbda) / 2
    v1 = (l + u) / 2
    v2 = (l - u) / 2
    a = (s1 * (l0 + u0) + s2 * (l0 - u0) + mu) / 2
    c = (s2 * (l0 + u0) + s1 * (l0 - u0) - mu) / 2
    return np.stack([v1, a, v2, c], axis=1).astype(np.float32)  # [N_IN, 4]


def kernel(weight, bias, bounds, bounds0, beta, lmbda, mu):
    nc = get_nc()
    vecs = _host_vectors(bounds, bounds0, beta, lmbda, mu)

    wt_full = np.ascontiguousarray(weight.T)  # [N_IN, N_OUT]
    in_maps = []
    for i in range(NCORES):
        in_maps.append({
            "wt": np.ascontiguousarray(wt_full[:, i * S:(i + 1) * S]),
            "vecs": vecs,
        })

    res = run_bass_kernel_spmd(nc, in_maps, core_ids=list(range(NCORES)))

    lower = np.empty(N_OUT, np.float32)
    upper = np.empty(N_OUT, np.float32)
    for i in range(NCORES):
        yz = res.results[i]["yz"]
        y1, z2 = yz[0, :S], yz[0, S:]
        ya, zc = yz[1, :S], yz[1, S:]
        b = bias[i * S:(i + 1) * S]
        lower[i * S:(i + 1) * S] = np.maximum(y1 + z2, ya + zc) + b
        upper[i * S:(i + 1) * S] = np.minimum(y1 - z2, ya - zc) + b
    return np.stack([lower, upper], axis=0)


# revision 10
# speedup vs baseline: 1.6327x; 1.6327x over previous
"""DeepPoly affine transformer — Trainium2 Bass kernel (8 NeuronCores).

Math: with A = |W|, and beta,lmbda >= 0 (so |Ml|,|Mu| are linear in W,|W|):
  lower     = W@v1 + A@v2 + b         v1 = (l+u)/2,  v2 = (l-u)/2
  upper     = W@v1 - A@v2 + b
  new_lower = W@a  + A@c  + b         s1 = (beta+lmbda)/2, s2 = (beta-lmbda)/2
  new_upper = W@a  - A@c  + b         a  = (s1*(l0+u0) + s2*(l0-u0) + mu)/2
                                      c  = (s2*(l0+u0) + s1*(l0-u0) - mu)/2
  out_lower = max(lower, new_lower);  out_upper = min(upper, new_upper)

Each core gets a 512-row slice of W (row-sharded over n_out), host-transposed
to W^T and cast to fp16 (10-bit mantissa; W ~ N(0, 0.05^2) fits comfortably),
stored partition-major [128, 64, 512] so every DMA reads long contiguous runs.
The PE streams W^T and |W^T| (2-byte operands stream at full rate, 4-byte at
half rate) against 2 stationary vectors each, accumulating in fp32 PSUM:
yz = [W@v1, W@a | A@v2, A@c]. The O(n_out) final combine runs on host.
"""

import numpy as np

import concourse.mybir as mybir
import concourse.tile as tile
from concourse import bacc
from concourse.bass_utils import run_bass_kernel_spmd

N_OUT, N_IN = 4096, 8192
NCORES = 8
S = N_OUT // NCORES  # 512 output rows per core
P = 128
KT = N_IN // P       # 64 k-tiles of 128
KSUP = 8             # k-tiles per DMA supertile (1 MiB fp16 loads)
F32 = mybir.dt.float32
F16 = mybir.dt.float16

_CACHE = {}


def _build_nc():
    nc = bacc.Bacc("TRN2", target_bir_lowering=False, debug=False,
                   num_devices=NCORES)
    # partition-major: wt[p, ki, n] = W^T[ki*128 + p, n]
    wt = nc.dram_tensor("wt", [P, KT, S], F16, kind="ExternalInput").ap()
    vecs = nc.dram_tensor("vecs", [P, KT, 4], F16, kind="ExternalInput").ap()
    yz = nc.dram_tensor("yz", [2, 2 * S], F32, kind="ExternalOutput").ap()

    with tile.TileContext(nc) as tc:
        with (
            tc.tile_pool(name="wp", bufs=3) as wpool,
            tc.tile_pool(name="apool", bufs=3) as apool,
            tc.tile_pool(name="vp", bufs=1) as vpool,
            tc.tile_pool(name="op", bufs=1) as opool,
            tc.tile_pool(name="ps", bufs=1, space="PSUM") as pspool,
        ):
            v_sb = vpool.tile([P, KT, 4], F16)
            nc.sync.dma_start(v_sb[:], vecs)

            psum_y = pspool.tile([2, S], F32, tag="Y")
            psum_z = pspool.tile([2, S], F32, tag="Z")

            for si in range(KT // KSUP):
                w_sb = wpool.tile([P, KSUP, S], F16, tag="w")
                nc.sync.dma_start(w_sb[:], wt[:, si * KSUP:(si + 1) * KSUP, :])
                a_sb = apool.tile([P, KSUP, S], F16, tag="a")
                # fp16 abs = clear the sign bit (abs_max isn't a valid
                # TensorScalar ALU op)
                nc.vector.tensor_scalar(
                    a_sb.bitcast(mybir.dt.int16), w_sb.bitcast(mybir.dt.int16),
                    0x7FFF, None, mybir.AluOpType.bitwise_and)
                for j in range(KSUP):
                    ki = si * KSUP + j
                    nc.tensor.matmul(
                        psum_y[:],
                        v_sb[:, ki, 0:2],
                        w_sb[:, j, :],
                        start=(ki == 0), stop=(ki == KT - 1))
                    nc.tensor.matmul(
                        psum_z[:],
                        v_sb[:, ki, 2:4],
                        a_sb[:, j, :],
                        start=(ki == 0), stop=(ki == KT - 1))

            # pack along free dim: row0 = [y1 | z2], row1 = [ya | zc]
            out_sb = opool.tile([2, 2 * S], F32)
            nc.vector.tensor_copy(out=out_sb[:, 0:S], in_=psum_y[:])
            nc.vector.tensor_copy(out=out_sb[:, S:2 * S], in_=psum_z[:])
            nc.sync.dma_start(yz, out_sb[:])

    nc.compile()
    return nc


def get_nc():
    if "nc" not in _CACHE:
        _CACHE["nc"] = _build_nc()
    return _CACHE["nc"]


def _host_vectors(bounds, bounds0, beta, lmbda, mu):
    l, u = bounds[0].astype(np.float64), bounds[1].astype(np.float64)
    l0, u0 = bounds0[0].astype(np.float64), bounds0[1].astype(np.float64)
    beta = beta.astype(np.float64)
    lmbda = lmbda.astype(np.float64)
    mu = mu.astype(np.float64)
    s1 = (beta + lmbda) / 2
    s2 = (beta - lmbda) / 2
    v1 = (l + u) / 2
    v2 = (l - u) / 2
    a = (s1 * (l0 + u0) + s2 * (l0 - u0) + mu) / 2
    c = (s2 * (l0 + u0) + s1 * (l0 - u0) - mu) / 2
    vecs = np.stack([v1, a, v2, c], axis=1)              # [N_IN, 4]
    return np.ascontiguousarray(
        vecs.reshape(KT, P, 4).transpose(1, 0, 2)).astype(np.float16)


def build_in_maps(weight, bounds, bounds0, beta, lmbda, mu):
    vecs = _host_vectors(bounds, bounds0, beta, lmbda, mu)
    in_maps = []
    for i in range(NCORES):
        wt = weight[i * S:(i + 1) * S].T                 # [N_IN, S] view
        wt = np.ascontiguousarray(
            wt.reshape(KT, P, S).transpose(1, 0, 2)).astype(np.float16)
        in_maps.append({"wt": wt, "vecs": vecs})
    return in_maps


def kernel(weight, bias, bounds, bounds0, beta, lmbda, mu):
    nc = get_nc()
    in_maps = build_in_maps(weight, bounds, bounds0, beta, lmbda, mu)
    res = run_bass_kernel_spmd(nc, in_maps, core_ids=list(range(NCORES)))

    lower = np.empty(N_OUT, np.float32)
    upper = np.empty(N_OUT, np.float32)
    for i in range(NCORES):
        yz = res.results[i]["yz"]
        y1, z2 = yz[0, :S], yz[0, S:]
        ya, zc = yz[1, :S], yz[1, S:]
        b = bias[i * S:(i + 1) * S]
        lower[i * S:(i + 1) * S] = np.maximum(y1 + z2, ya + zc) + b
        upper[i * S:(i + 1) * S] = np.minimum(y1 - z2, ya - zc) + b
    return np.stack([lower, upper], axis=0)


# revision 12
# speedup vs baseline: 1.7589x; 1.0773x over previous
"""DeepPoly affine transformer — Trainium2 Bass kernel (8 NeuronCores).

Math: with A = |W|, and beta,lmbda >= 0 (so |Ml|,|Mu| are linear in W,|W|):
  lower     = W@v1 + A@v2 + b         v1 = (l+u)/2,  v2 = (l-u)/2
  upper     = W@v1 - A@v2 + b
  new_lower = W@a  + A@c  + b         s1 = (beta+lmbda)/2, s2 = (beta-lmbda)/2
  new_upper = W@a  - A@c  + b         a  = (s1*(l0+u0) + s2*(l0-u0) + mu)/2
                                      c  = (s2*(l0+u0) + s1*(l0-u0) - mu)/2
  out_lower = max(lower, new_lower);  out_upper = min(upper, new_upper)

Each core gets a 512-row slice of W (row-sharded over n_out), host-transposed
to W^T and cast to fp16 (10-bit mantissa; W ~ N(0, 0.05^2) fits comfortably),
stored partition-major [128, 64, 512] so every DMA reads long contiguous runs.
The PE streams W^T and |W^T| (2-byte operands stream at full rate, 4-byte at
half rate) against 2 stationary vectors each, accumulating in fp32 PSUM:
yz = [W@v1, W@a | A@v2, A@c]. The O(n_out) final combine runs on host.
"""

import numpy as np

import concourse.mybir as mybir
import concourse.tile as tile
from concourse import bacc
from concourse.bass_utils import run_bass_kernel_spmd

N_OUT, N_IN = 4096, 8192
NCORES = 8
S = N_OUT // NCORES  # 512 output rows per core
P = 128
KT = N_IN // P       # 64 k-tiles of 128
KSUP = 8             # k-tiles per DMA supertile (1 MiB fp16 loads)
F32 = mybir.dt.float32
F16 = mybir.dt.float16

_CACHE = {}


def _build_nc():
    nc = bacc.Bacc("TRN2", target_bir_lowering=False, debug=False,
                   num_devices=NCORES)
    # partition-major: wt[p, ki, n] = W^T[ki*128 + p, n]
    wt = nc.dram_tensor("wt", [P, KT, S], F16, kind="ExternalInput").ap()
    vecs = nc.dram_tensor("vecs", [P, KT, 4], F16, kind="ExternalInput").ap()
    yz = nc.dram_tensor("yz", [2, 2 * S], F32, kind="ExternalOutput").ap()

    # first supertiles are small so the PE starts early; rest amortize DMA
    sups = [2, 6] + [KSUP] * ((KT - 8) // KSUP)
    assert sum(sups) == KT

    with tile.TileContext(nc) as tc:
        with (
            tc.tile_pool(name="wp", bufs=4) as wpool,
            tc.tile_pool(name="apool", bufs=4) as apool,
            tc.tile_pool(name="vp", bufs=1) as vpool,
            tc.tile_pool(name="op", bufs=1) as opool,
            tc.tile_pool(name="ps", bufs=1, space="PSUM") as pspool,
        ):
            v_sb = vpool.tile([P, KT, 4], F16)
            nc.sync.dma_start(v_sb[:], vecs)

            psum_y = pspool.tile([2, S], F32, tag="Y")
            # Z accumulates at PSUM base partition 32 so its matmuls land on
            # a different PE column-group and run concurrently with Y's.
            psum_z = pspool.tile([34, S], F32, tag="Z", name="psum_z")[32:34]

            ki = 0
            for si, ksup in enumerate(sups):
                w_sb = wpool.tile([P, KSUP, S], F16, tag="w", name="w_sb")[:, :ksup, :]
                nc.sync.dma_start(w_sb[:], wt[:, ki:ki + ksup, :])
                a_sb = apool.tile([P, KSUP, S], F16, tag="a", name="a_sb")[:, :ksup, :]
                # fp16 abs = clear the sign bit (abs_max isn't a valid
                # TensorScalar ALU op)
                nc.vector.tensor_scalar(
                    a_sb.bitcast(mybir.dt.int16), w_sb.bitcast(mybir.dt.int16),
                    0x7FFF, None, mybir.AluOpType.bitwise_and)
                for j in range(ksup):
                    nc.tensor.matmul(
                        psum_y[:],
                        v_sb[:, ki, 0:2],
                        w_sb[:, j, :],
                        start=(ki == 0), stop=(ki == KT - 1),
                        tile_position=(0, 0))
                    nc.tensor.matmul(
                        psum_z[:],
                        v_sb[:, ki, 2:4],
                        a_sb[:, j, :],
                        start=(ki == 0), stop=(ki == KT - 1),
                        tile_position=(0, 32))
                    ki += 1

            # pack along free dim: row0 = [y1 | z2], row1 = [ya | zc]
            out_sb = opool.tile([2, 2 * S], F32)
            nc.scalar.copy(out=out_sb[:, 0:S], in_=psum_y[:])
            nc.vector.tensor_copy(out=out_sb[:, S:2 * S], in_=psum_z[:])
            nc.sync.dma_start(yz, out_sb[:])

    nc.compile()
    return nc


def get_nc():
    if "nc" not in _CACHE:
        _CACHE["nc"] = _build_nc()
    return _CACHE["nc"]


def _host_vectors(bounds, bounds0, beta, lmbda, mu):
    l, u = bounds[0].astype(np.float64), bounds[1].astype(np.float64)
    l0, u0 = bounds0[0].astype(np.float64), bounds0[1].astype(np.float64)
    beta = beta.astype(np.float64)
    lmbda = lmbda.astype(np.float64)
    mu = mu.astype(np.float64)
    s1 = (beta + lmbda) / 2
    s2 = (beta - lmbda) / 2
    v1 = (l + u) / 2
    v2 = (l - u) / 2
    a = (s1 * (l0 + u0) + s2 * (l0 - u0) + mu) / 2
    c = (s2 * (l0 + u0) + s1 * (l0 - u0) - mu) / 2
    vecs = np.stack([v1, a, v2, c], axis=1)              # [N_IN, 4]
    return np.ascontiguousarray(
        vecs.reshape(KT, P, 4).transpose(1, 0, 2)).astype(np.float16)


def build_in_maps(weight, bounds, bounds0, beta, lmbda, mu):
    vecs = _host_vectors(bounds, bounds0, beta, lmbda, mu)
    in_maps = []
    for i in range(NCORES):
        wt = weight[i * S:(i + 1) * S].T                 # [N_IN, S] view
        wt = np.ascontiguousarray(
            wt.reshape(KT, P, S).transpose(1, 0, 2)).astype(np.float16)
        in_maps.append({"wt": wt, "vecs": vecs})
    return in_maps


def kernel(weight, bias, bounds, bounds0, beta, lmbda, mu):
    nc = get_nc()
    in_maps = build_in_maps(weight, bounds, bounds0, beta, lmbda, mu)
    res = run_bass_kernel_spmd(nc, in_maps, core_ids=list(range(NCORES)))

    lower = np.empty(N_OUT, np.float32)
    upper = np.empty(N_OUT, np.float32)
    for i in range(NCORES):
        yz = res.results[i]["yz"]
        y1, z2 = yz[0, :S], yz[0, S:]
        ya, zc = yz[1, :S], yz[1, S:]
        b = bias[i * S:(i + 1) * S]
        lower[i * S:(i + 1) * S] = np.maximum(y1 + z2, ya + zc) + b
        upper[i * S:(i + 1) * S] = np.minimum(y1 - z2, ya - zc) + b
    return np.stack([lower, upper], axis=0)
